# revision 14
# baseline (speedup 1.0000x reference)
"""Trainium2 Bass kernel for nn_BaseAttention (causal MHA, b=2, n=2048, d=1024, 16 heads).

Sharding (8 cores): core c handles batch c//4 and heads 4*(c%4)..4*(c%4)+3.
- W_q/W_k/W_v column-sharded (256 cols/core), W_o row-sharded (256 rows/core).
- Each core computes a partial output [2048, 1024] in fp32; host sums the 4
  partials per batch (row-parallel out-projection) and stacks the 2 batches.

Per-core kernel (bf16 data path, fp32 PSUM accumulation; ~160 us HW exec,
rel err ~3.5e-3 vs the fp32 reference):
  - x is transposed + bf16-cast on the host (shared by the 4 cores of each
    batch) so x^T loads as full-bandwidth contiguous copy DMAs; weights are
    pre-laid-out to their SBUF layouts on the host as well.
  - Q^T/K^T projections emitted transposed; V natural with a ones column per
    head ([V|1] trick: the AV matmul yields ctx^T on psum partitions 0..63
    and the softmax row-sum at partition 64 in one pass).
  - attention per (head-pair, q-tile j): S^T = K_h @ Q_h^T on PE (even/odd
    heads on disjoint PE row-halves), exp on ACT over [128,1024] psum pairs
    with the 1/8 scale fused, causal mask via gpsimd affine_select
    (exp-then-zero; S/AV/mask all narrowed to the valid causal q-range), AV
    pipelined one i-pair behind S, normalization via DVE
    reciprocal_approx_fast (SBUF input only - broken from PSUM) + gpsimd
    partition_broadcast + DVE multiply.
  - projection work of round g+1 (or out-projection chunks in the last
    round) is woven between attention steps so the in-order PE queue never
    idles while ACT catches up (keeps HAM warm).
  - out-projection from ctx^T; bias added by a DVE tensor_add against a
    pre-broadcast b_o tile during the PSUM drain.
  - copy-DMAs and transpose-DMAs must not interleave (xbar mode transitions
    serialize); all DMAs here are copies. DMA issue is ~0.6us each on a
    sequencer, so transfers are consolidated into few instructions and
    split across the sync + scalar HWDGE queues.
"""
import sys, types

sys.path.insert(0, "/opt/trn_rl_repo")


def _install_ntff_shim():
    # antenv.axon_hooks is absent in this image; register the NTFF profile
    # hook via ctypes so run_bass_kernel_spmd(trace=True) works under axon.
    if "antenv.axon_hooks" in sys.modules:
        return
    try:
        sys.path.insert(0, "/root/.axon_site")
        from trn_agent_boot.trn_boot import _ntff_profile_via_ctypes

        hook = _ntff_profile_via_ctypes("/opt/axon/libaxon_pjrt.so")
        mod = types.ModuleType("antenv.axon_hooks")
        mod.get_axon_ntff_profile_hook = lambda: hook
        mod.set_axon_ntff_profile_hook = lambda h: None
        sys.modules["antenv.axon_hooks"] = mod
    except Exception:
        pass


_install_ntff_shim()

import numpy as np
import ml_dtypes
import concourse.bass as bass
import concourse.mybir as mybir
import concourse.tile as tile
from concourse import bacc
from concourse.bass_utils import run_bass_kernel_spmd
from contextlib import ExitStack

f32 = mybir.dt.float32
bf16 = mybir.dt.bfloat16
EXP = mybir.ActivationFunctionType.Exp

SEQ = 2048          # sequence length
DIN = 1024          # model dim (8 chunks of 128)
QC = 256            # q/k/v cols per core (4 heads x 64)
HD = 64             # head dim
NH = 4              # heads per core
NG = 4              # row groups of 512
VST = NH * 65       # Vones stride per row chunk (4 heads x (64 V + 1 ones))

TRACE = False
LAST_RESULTS = None
AV_SPLIT = False


def build_nc():
    nc = bacc.Bacc()
    # x pre-laid-out on host to the SBUF layout, g-major:
    # x_d[p, g*4096 + c*512 + r] = x[g*512+r, c*128+p] — each g-block DMAs as
    # one contiguous [128, 4096] copy (128 x 8KB descriptors, cheap issue).
    x_d = nc.dram_tensor("x", [128, 8 * SEQ], bf16, kind="ExternalInput")
    wq_d = nc.dram_tensor("wq", [128, 8 * QC], bf16, kind="ExternalInput")
    wk_d = nc.dram_tensor("wk", [128, 8 * QC], bf16, kind="ExternalInput")
    wv_d = nc.dram_tensor("wv", [128, 8 * QC], bf16, kind="ExternalInput")
    wo_d = nc.dram_tensor("wo", [128, 2 * DIN], bf16, kind="ExternalInput")
    out_d = nc.dram_tensor("out", [SEQ, DIN], bf16, kind="ExternalOutput")
    # u0-half partials of the last four row-chunks (q 1536:2048) drain here
    # during u=1's attention; host adds them to out rows 1536:2048.
    out2_d = nc.dram_tensor("out2", [512, DIN], bf16, kind="ExternalOutput")

    with tile.TileContext(nc, pool_alloc_mode="queue") as tc, ExitStack() as ctx:
        cst = ctx.enter_context(tc.tile_pool(name="cst", bufs=1))
        wr = ctx.enter_context(tc.tile_pool(name="wr", bufs=1))
        big = ctx.enter_context(tc.tile_pool(name="big", bufs=1))
        ptp = ctx.enter_context(tc.tile_pool(name="ptp", bufs=8))
        nrm = ctx.enter_context(tc.tile_pool(name="nrm", bufs=3))
        ob = ctx.enter_context(tc.tile_pool(name="ob", bufs=6))
        ps = ctx.enter_context(tc.tile_pool(name="ps", bufs=1, space="PSUM"))

        # ---- DMAs, ordered by first use (weights pre-laid-out on host).
        # weights issue on the scalar HWDGE queue, x on sync: parallel issue.
        # The first interleaved Q/K chain (c=0..3 of t=0) only needs
        # wq/wk cols 0:512 and x g0 c0..3, so those 128KB chunks go first.
        xT = big.tile([128, 8 * SEQ], bf16)
        wq_sb = wr.tile([128, 8 * QC], bf16, name="wq_sb")
        wk_sb = wr.tile([128, 8 * QC], bf16, name="wk_sb")
        wv_sb = wr.tile([128, 8 * QC], bf16, name="wv_sb")
        nc.scalar.dma_start(wq_sb[:, 0:512], wq_d[:, 0:512])
        nc.sync.dma_start(xT[:, 0:512], x_d[:, 0:512])            # g0, c=0
        nc.scalar.dma_start(wk_sb[:, 0:512], wk_d[:, 0:512])
        nc.sync.dma_start(xT[:, 512:2048], x_d[:, 512:2048])      # g0, c=1..3
        nc.scalar.dma_start(wq_sb[:, 512:1024], wq_d[:, 512:1024])
        nc.scalar.dma_start(wk_sb[:, 512:1024], wk_d[:, 512:1024])
        nc.sync.dma_start(xT[:, 2048:4096], x_d[:, 2048:4096])    # g0, c=4..7
        nc.scalar.dma_start(wq_sb[:, 1024:], wq_d[:, 1024:])
        nc.scalar.dma_start(wk_sb[:, 1024:], wk_d[:, 1024:])
        nc.scalar.dma_start(wv_sb[:], wv_d[:])
        for g in range(1, NG):
            nc.sync.dma_start(
                xT[:, g * 4096:(g + 1) * 4096],
                x_d[:, g * 4096:(g + 1) * 4096],
            )
        wo_sb = cst.tile([128, 2 * DIN], bf16)
        nc.scalar.dma_start(wo_sb[:], wo_d[:])

        # ---- persistent activations ----
        qt_sb = [big.tile([128, SEQ], bf16, name=f"qt{t}") for t in range(2)]
        kt_sb = [big.tile([128, SEQ], bf16, name=f"kt{t}") for t in range(2)]
        vones = big.tile([128, 16 * VST], bf16)
        ctxt = [big.tile([128, SEQ], bf16, name=f"ctxt{t}") for t in range(2)]

        vview = vones.rearrange("p (r h e) -> p r h e", h=NH, e=65)
        nc.vector.memset(vview[:, :, :, 64], 1.0)

        # prime the gpsimd partition_broadcast ucode library at startup:
        # the first broadcast otherwise triggers a ~7us LIBRARY_RELOAD in
        # the middle of the first normalize chain, stalling every engine
        prime_src = cst.tile([1, 16], f32)
        prime_dst = cst.tile([2, 16], f32)
        nc.vector.memset(prime_src[:], 1.0)
        nc.gpsimd.partition_broadcast(prime_dst[:], prime_src[:])

        # ---- emission helpers ----
        # Q and K chains for one (g, t) interleave across two psum banks
        # (tag "b" slots): back-to-back accumulating matmuls into the SAME
        # psum bank serialize the PE (~275 ns/MM vs ~225 interleaved).
        # Emitted as two filler-sized halves sharing the live psum tiles.
        def emit_qk_pair(g, t, part, st):
            if part == 0:
                st["q"] = ps.tile([128, 512], f32, tag="b", bufs=2, name="prjq")
                st["k"] = ps.tile([128, 512], f32, tag="b", bufs=2, name="prjk")
            c0 = part * 4
            for c in range(c0, c0 + 4):
                for wt, pj in ((wq_sb, st["q"]), (wk_sb, st["k"])):
                    nc.tensor.matmul(
                        pj[:],
                        wt[:, t * 1024 + c * 128: t * 1024 + c * 128 + 128],
                        xT[:, g * 4096 + c * 512: g * 4096 + c * 512 + 512],
                        start=(c == 0),
                        stop=(c == 7),
                    )
            if part == 1:
                nc.vector.tensor_copy(qt_sb[t][:, g * 512:(g + 1) * 512], st["q"][:])
                nc.vector.tensor_copy(kt_sb[t][:, g * 512:(g + 1) * 512], st["k"][:])

        def emit_v_pair(g, rc0, part, st):
            if part == 0:
                st["a"] = ps.tile([128, 256], f32, tag="b", bufs=2, name="vpsa")
                st["b"] = ps.tile([128, 256], f32, tag="b", bufs=2, name="vpsb")
            c0 = part * 4
            for c in range(c0, c0 + 4):
                for rc, key in ((rc0, "a"), (rc0 + 1, "b")):
                    nc.tensor.matmul(
                        st[key][:],
                        xT[:, g * 4096 + c * 512 + rc * 128: g * 4096 + c * 512 + rc * 128 + 128],
                        wv_sb[:, c * QC:(c + 1) * QC],
                        start=(c == 0),
                        stop=(c == 7),
                    )
            if part == 1:
                for rc, key in ((rc0, "a"), (rc0 + 1, "b")):
                    nc.vector.tensor_copy(
                        vview[:, 4 * g + rc, :, 0:64],
                        st[key][:].rearrange("p (h e) -> p h e", e=HD),
                    )

        def drain_out(rc, n, ops, split=1):
            # bias is added on the host (rides the partial-sum); cast drain.
            # In-window chunks stay on the sync queue (a DIRECT2D issue on
            # the scalar queue blocks exp and stalls the attention pipe);
            # tail chunks split across sync+scalar (ACT is idle by then) so
            # a late chunk never serializes ~10us on one DMA engine.
            osb = ob.tile([128, 512], bf16, tag="o", name="osb")
            nc.vector.tensor_copy(osb[:], ops[:])
            step = 128 // split
            for s in range(split):
                q = nc.scalar if (split > 1 and s % 2) else nc.sync
                q.dma_start(
                    out_d[rc * 128 + s * step: rc * 128 + (s + 1) * step,
                          n * 512:(n + 1) * 512],
                    osb[s * step:(s + 1) * step, :],
                )

        # two out-proj chunks interleaved over two psum banks
        def emit_outproj_pair(rcn_a, rcn_b, tag="b", split=1):
            opsa = ps.tile([128, 512], f32, tag=tag, bufs=2, name="opsa")
            opsb = ps.tile([128, 512], f32, tag=tag, bufs=2, name="opsb")
            for u in range(2):
                for (rc, n), ops in ((rcn_a, opsa), (rcn_b, opsb)):
                    nc.tensor.matmul(
                        ops[:],
                        ctxt[u][:, rc * 128:(rc + 1) * 128],
                        wo_sb[:, u * DIN + n * 512: u * DIN + n * 512 + 512],
                        start=(u == 0),
                        stop=(u == 1),
                    )
            for (rc, n), ops in ((rcn_a, opsa), (rcn_b, opsb)):
                drain_out(rc, n, ops, split=split)

        def proj_chunks(g):
            for t in range(2):
                st = {}
                yield lambda t=t, st=st: emit_qk_pair(g, t, 0, st)
                yield lambda t=t, st=st: emit_qk_pair(g, t, 1, st)
            for rc0 in (0, 2):
                st = {}
                yield lambda rc0=rc0, st=st: emit_v_pair(g, rc0, 0, st)
                yield lambda rc0=rc0, st=st: emit_v_pair(g, rc0, 1, st)

        # ---- round 0 projections up-front ----
        for f in proj_chunks(0):
            f()

        # ---- main rounds: attention(j=g) woven with proj(g+1)/outproj ----
        for g in range(NG):
            j = g
            imax = 4 * j + 3
            npair = (imax + 1) // 2
            if g < NG - 1:
                filler = list(proj_chunks(g + 1))
            else:
                # last round: weave the 12 ready out-proj chunk-pairs; 8 go
                # to u=0's ACT-bound stretch, 4 + the rc12-15 u0-halves
                # (pushed after u0's normalize) to u=1's.
                filler = [
                    (lambda rc=rc: emit_outproj_pair((rc, 0), (rc, 1)))
                    for rc in range(12)
                ]
            steps_total = 2 * npair
            fill_i = 0
            step = 0

            def emit_u0_half(rc):
                # u0 partial of out rows rc*128..+128 -> out2 (host adds)
                opsa = ps.tile([128, 512], f32, tag="b", bufs=2, name="opsa")
                opsb = ps.tile([128, 512], f32, tag="b", bufs=2, name="opsb")
                for n, ops in ((0, opsa), (1, opsb)):
                    nc.tensor.matmul(
                        ops[:],
                        ctxt[0][:, rc * 128:(rc + 1) * 128],
                        wo_sb[:, n * 512: n * 512 + 512],
                        start=True,
                        stop=True,
                    )
                for n, ops in ((0, opsa), (1, opsb)):
                    osb = ob.tile([128, 512], bf16, tag="o", name="osb")
                    nc.vector.tensor_copy(osb[:], ops[:])
                    q = nc.scalar if n else nc.sync
                    q.dma_start(
                        out2_d[(rc - 12) * 128:(rc - 11) * 128,
                               n * 512:(n + 1) * 512],
                        osb[:],
                    )

            for u in range(2):           # head pair u: heads 2u, 2u+1
                avs = [ps.tile([65, 512], f32, tag="av", bufs=2, name=f"av{p}")
                       for p in range(2)]
                pts = [[], []]           # per parity: list of [128,1024] pair tiles
                for ip in range(npair):
                    i0 = 2 * ip
                    cur = []
                    for p in range(2):
                        sps = ps.tile([128, 1024], f32, tag="a", bufs=2, name="sps")
                        cur.append(sps)
                    for half in range(2):
                        i = i0 + half
                        off = max(0, 128 * i - 512 * j)
                        for p in range(2):
                            o = p * 64
                            nc.tensor.matmul(
                                cur[p][:, half * 512 + off:(half + 1) * 512],
                                kt_sb[u][o:o + 64, i * 128:(i + 1) * 128],
                                qt_sb[u][o:o + 64, j * 512 + off:(j + 1) * 512],
                                start=True,
                                stop=True,
                            )
                    off0 = max(0, 128 * i0 - 512 * j)
                    off1 = max(0, 128 * (i0 + 1) - 512 * j)
                    for p in range(2):
                        pt = ptp.tile([128, 1024], bf16, tag="pt", name="pt")
                        if off0 == 0:
                            nc.scalar.activation(pt[:], cur[p][:], EXP, scale=0.125)
                        else:
                            # diagonal pair: skip the non-causal column ranges
                            nc.scalar.activation(
                                pt[:, off0:512], cur[p][:, off0:512], EXP,
                                scale=0.125)
                            nc.scalar.activation(
                                pt[:, 512 + off1:1024], cur[p][:, 512 + off1:1024],
                                EXP, scale=0.125)
                        for half in range(2):
                            i = i0 + half
                            if i >= 4 * j:
                                off = 128 * i - 512 * j
                                w = min(128, 512 - off)
                                nc.gpsimd.affine_select(
                                    out=pt[:, half * 512 + off: half * 512 + off + w],
                                    in_=pt[:, half * 512 + off: half * 512 + off + w],
                                    compare_op=mybir.AluOpType.is_ge,
                                    fill=0.0,
                                    base=0,
                                    channel_multiplier=-1,
                                    pattern=[[1, w]],
                                )
                        pts[p].append(pt)
                    if ip >= 1:
                        kp = ip - 1
                        # half-then-parity order alternates the two avs psum
                        # banks between consecutive matmuls (same-bank
                        # accumulation runs ~336 ns/MM vs ~225 alternated)
                        for half in range(2):
                            k = 2 * kp + half
                            off = max(0, 128 * k - 512 * j)
                            for p in range(2):
                                h = 2 * u + p
                                nc.tensor.matmul(
                                    avs[p][:, off:512],
                                    vones[:, k * VST + h * 65: k * VST + h * 65 + 65],
                                    pts[p][kp][:, half * 512 + off:(half + 1) * 512],
                                    start=(k == 0),
                                    stop=False,
                                )
                    step += 1
                    want = (len(filler) * step) // steps_total
                    while fill_i < want:
                        filler[fill_i]()
                        fill_i += 1
                # tail AVs for the last pair + per-parity normalize
                kp = npair - 1
                for half in range(2):
                    k = 2 * kp + half
                    off = max(0, 128 * k - 512 * j)
                    for p in range(2):
                        h = 2 * u + p
                        nc.tensor.matmul(
                            avs[p][:, off:512],
                            vones[:, k * VST + h * 65: k * VST + h * 65 + 65],
                            pts[p][kp][:, half * 512 + off:(half + 1) * 512],
                            start=(k == 0),
                            stop=(half == 1),
                        )
                for p in range(2):
                    o = p * 64
                    rsrow = nrm.tile([1, 512], f32, tag="rsrow", name="rsrow")
                    nc.vector.tensor_copy(rsrow[:], avs[p][64:65, :])
                    rinv = nrm.tile([1, 512], f32, tag="rinv", name="rinv")
                    nc.vector.reciprocal_approx_fast(rinv[:], rsrow[:])
                    bcast = nrm.tile([64, 512], f32, tag="bcast", name="bcast")
                    nc.gpsimd.partition_broadcast(bcast[:], rinv[:])
                    nc.vector.tensor_mul(
                        ctxt[u][o:o + 64, j * 512:(j + 1) * 512],
                        avs[p][0:64, :],
                        bcast[:],
                    )
                if g == NG - 1 and u == 0:
                    # ctxt[0] q-window j=3 just completed: the u0-halves of
                    # the last four out rows become u=1-stretch filler.
                    filler.extend(
                        (lambda rc=rc: emit_u0_half(rc)) for rc in range(12, 16)
                    )
            while fill_i < len(filler):
                filler[fill_i]()
                fill_i += 1

        # ---- tail: u=1 halves of the last four row-chunks. Casts alternate
        # DVE / ACT (ACT's exp stream is finished by now) and the final DMAs
        # split across both queues so no single engine serializes the drain.
        tail_ps = []
        for rc in range(12, 16):
            for n in range(2):
                tag = "av" if (rc + n) % 2 else "b"
                ops = ps.tile([128, 512], f32, tag=tag, bufs=2, name="ops")
                nc.tensor.matmul(
                    ops[:],
                    ctxt[1][:, rc * 128:(rc + 1) * 128],
                    wo_sb[:, DIN + n * 512: DIN + n * 512 + 512],
                    start=True,
                    stop=True,
                )
                tail_ps.append((rc, n, ops))
        for k, (rc, n, ops) in enumerate(tail_ps):
            osb = ob.tile([128, 512], bf16, tag="o", name="osb")
            if k % 2:
                nc.scalar.activation(osb[:], ops[:], mybir.ActivationFunctionType.Copy)
            else:
                nc.vector.tensor_copy(osb[:], ops[:])
            for s in range(2):
                q = nc.scalar if (k + s) % 2 else nc.sync
                q.dma_start(
                    out_d[rc * 128 + s * 64: rc * 128 + (s + 1) * 64,
                          n * 512:(n + 1) * 512],
                    osb[s * 64:(s + 1) * 64, :],
                )

    nc.compile()
    return nc


_NC = None


def _get_nc():
    global _NC
    if _NC is None:
        _NC = build_nc()
    return _NC


def kernel(x, W_q, W_k, W_v, W_o, b_o):
    global LAST_RESULTS
    nc = _get_nc()
    bf = ml_dtypes.bfloat16
    x = np.asarray(x, np.float32).astype(bf)
    # pre-lay-out per batch to the SBUF layout (shared by 4 cores per batch):
    # x2[p, g*4096 + c*512 + r] = x[g*512+r, c*128+p]
    xT = [np.ascontiguousarray(
        x[bi].reshape(4, 512, 8, 128).transpose(3, 0, 2, 1).reshape(128, 8 * SEQ))
        for bi in range(2)]
    W_q = np.asarray(W_q, np.float32).astype(bf)
    W_k = np.asarray(W_k, np.float32).astype(bf)
    W_v = np.asarray(W_v, np.float32).astype(bf)
    W_o = np.asarray(W_o, np.float32).astype(bf)
    b_o32 = np.asarray(b_o, np.float32).reshape(1, DIN)

    def lay_w(w, sl):   # [1024, 256] shard -> [128, 2048]: t[p, t*1024+c*128+n] = w[c*128+p, sl][t*128+n]
        return np.ascontiguousarray(
            w[:, sl].reshape(8, 128, 2, 128).transpose(1, 2, 0, 3).reshape(128, 8 * QC))

    def lay_wv(w, sl):  # [1024, 256] shard -> [128, 8*256]: t[p, c*256+n] = w[c*128+p, sl][n]
        return np.ascontiguousarray(
            w[:, sl].reshape(8, 128, QC).transpose(1, 0, 2).reshape(128, 8 * QC))

    def lay_wo(w, sl):  # [256, 1024] shard -> [128, 2*1024]
        return np.ascontiguousarray(
            w[sl, :].reshape(2, 128, DIN).transpose(1, 0, 2).reshape(128, 2 * DIN))

    in_maps = []
    for c in range(8):
        bi, g = c // 4, c % 4
        sl = slice(g * QC, (g + 1) * QC)
        in_maps.append({
            "x": xT[bi],
            "wq": lay_w(W_q, sl),
            "wk": lay_w(W_k, sl),
            "wv": lay_wv(W_v, sl),
            "wo": lay_wo(W_o, sl),
        })

    res = run_bass_kernel_spmd(nc, in_maps, list(range(8)), trace=TRACE)
    LAST_RESULTS = res
    outs = []
    for r in res.results:
        o = np.asarray(r["out"], dtype=np.float32)
        o[1536:2048] += np.asarray(r["out2"], dtype=np.float32)
        outs.append(o)
    return np.stack([
        outs[0] + outs[1] + outs[2] + outs[3] + b_o32,
        outs[4] + outs[5] + outs[6] + outs[7] + b_o32,
    ])


if __name__ == "__main__":
    if "--compile-only" in sys.argv:
        import tempfile
        from concourse.bass_utils import compile_bass_kernel

        nc = build_nc()
        with tempfile.TemporaryDirectory() as td:
            print("walrus compiling...")
            neff = compile_bass_kernel(nc, td)
            print("COMPILE OK", neff)

